# revision 21
# baseline (speedup 1.0000x reference)
"""Trainium2 Bass kernel for nn_DFDgraph (gnn_message_passing).

Pipeline per batch element (one NeuronCore each, 8 total):
  x (2048, 288) --rfft-mag--> (2048, 145) --minmax+l2--> xn
  h = LN(relu(cat[xn @ Wd0, te_norm] @ We0))            (2048, 64)
  adj = relu((h * w) @ h^T)                             (2048, 2048)
  out = top10_row_mask(adj) / (rowsum_kept + 1e-5)

The rfft is two matmuls against a host-precomputed DFT cos|sin matrix
(288 x 290, ortho-normalized), keeping everything fp32 (f32r loses
~1e-3 relative accuracy, which flips top-k selections). Top-10 per row:
DVE max8 -> match_replace(0) -> max8 gives the 10 largest values
exactly; kept = (adj >= v10) * adj via scalar_tensor_tensor on GpSimd,
final scale 1/(sum_top10 + 1e-5) on ACT.

Phase 1 is processed in groups of 4 row-tiles so the per-group stages
(DFT -> normalize -> MLP -> LN -> transpose) pipeline across groups.
"""

import numpy as np
from contextlib import ExitStack

import concourse.bass as bass
import concourse.mybir as mybir
from concourse import bacc
from concourse import tile
from concourse import masks
from concourse.bass_utils import run_bass_kernel_spmd

F32 = mybir.dt.float32
F16 = mybir.dt.float16
AX = mybir.AxisListType
OP = mybir.AluOpType
AF = mybir.ActivationFunctionType

B, N, T, H, EMB, TOPK = 8, 2048, 288, 64, 24, 10
F = T // 2 + 1          # 145
P = 128                 # rows per tile
NT = N // P             # 16 tiles
G = 4                   # tiles per pipeline group
KC = 96                 # DFT contraction chunk (3 x 96 = 288)
NCORES = 8

_CACHE = {}


def _build(sel_engine="gpsimd"):
    nc = bacc.Bacc("TRN2", target_bir_lowering=False, debug=False,
                   num_devices=NCORES)
    x_d = nc.declare_dram_parameter("x", [N, T], F32, isOutput=False)
    te_d = nc.declare_dram_parameter("t_emb", [N, EMB], F32, isOutput=False)
    cc_d = nc.declare_dram_parameter("ccos", [T, F], F32, isOutput=False)
    cs_d = nc.declare_dram_parameter("csin", [T, F], F32, isOutput=False)
    wd_d = nc.declare_dram_parameter("wd0", [F, H], F32, isOutput=False)
    we_d = nc.declare_dram_parameter("we0", [H + EMB, H], F32, isOutput=False)
    w_d = nc.declare_dram_parameter("w", [H, 1], F32, isOutput=False)
    out_d = nc.declare_dram_parameter("out", [N, N], F32, isOutput=True)

    with tile.TileContext(nc) as tc, ExitStack() as ctx:
        const = ctx.enter_context(tc.tile_pool(name="const", bufs=1))
        ident = const.tile([P, P], F32)
        masks.make_identity(nc, ident[:])
        ccs_sb = const.tile([KC, 3, 2 * F], F32)
        for c in range(3):
            nc.sync.dma_start(ccs_sb[:, c, 0:F], cc_d[c * KC:(c + 1) * KC, :])
            nc.sync.dma_start(ccs_sb[:, c, F:2 * F], cs_d[c * KC:(c + 1) * KC, :])
        wd_a = const.tile([P, H], F32)
        wd_b = const.tile([F - P, H], F32)
        nc.sync.dma_start(wd_a[:], wd_d[0:P, :])
        nc.sync.dma_start(wd_b[:], wd_d[P:F, :])
        we_sb = const.tile([H + EMB, H], F32)
        nc.sync.dma_start(we_sb[:], we_d[:])
        w2_sb = const.tile([H, 1], F32)
        nc.sync.dma_start(w2_sb[:], w_d[:])

        # persistent phase-1 results (fp16 split of h^T for phase-2 matmuls)
        p1 = ctx.enter_context(tc.tile_pool(name="p1", bufs=1))
        hTd_sb = p1.tile([2 * H, N], F16)   # hT hi, duplicated on both halves
        hTl_sb = p1.tile([H, N], F16)       # hT lo
        lhsT_sb = p1.tile([2 * H, NT, P], F16)  # per-tile [hw_hi ; hw_lo]
        lhsTlo_sb = p1.tile([H, NT, P], F16)    # hw_lo staging (parts 0-63)
        # [P, NT] stats, persistent
        st = ctx.enter_context(tc.tile_pool(name="stats", bufs=1))
        mx_s = st.tile([P, NT], F32)
        mn_s = st.tile([P, NT], F32)
        rd_s = st.tile([P, NT], F32)
        ssx_s = st.tile([P, NT], F32)
        rnx_s = st.tile([P, NT], F32)
        mxt_s = st.tile([P, NT], F32)
        mnt_s = st.tile([P, NT], F32)
        rdt_s = st.tile([P, NT], F32)
        sst_s = st.tile([P, NT], F32)
        rnt_s = st.tile([P, NT], F32)
        sums_s = st.tile([P, NT], F32)
        mean_s = st.tile([P, NT], F32)
        ssh_s = st.tile([P, NT], F32)
        rstd_s = st.tile([P, NT], F32)
        mnr_s = st.tile([P, NT], F32)

        # group-cycled working buffers (bufs=2 -> group g+1 overlaps group g)
        p1ps = ExitStack()
        gp = p1ps.enter_context(tc.tile_pool(name="gp", bufs=2))
        ps_a = p1ps.enter_context(tc.tile_pool(name="ps_a", bufs=2, space="PSUM"))
        ps_b = p1ps.enter_context(tc.tile_pool(name="ps_b", bufs=2, space="PSUM"))

        for g in range(NT // G):
            t0 = g * G
            sl = slice(t0, t0 + G)
            # ---- stage A: load x/te, transpose, DFT, squares ----
            # ri2 = [re^2 | im^2]; its ACT accum gives S2 = sum(mag^2) free
            ri2 = gp.tile([P, G, 2 * F], F32, tag="ri2")
            te_g = gp.tile([P, G, EMB], F32, tag="te")
            for j in range(G):
                t = t0 + j
                x_t = gp.tile([P, T], F32, tag="x")
                nc.sync.dma_start(x_t[:], x_d[t * P:(t + 1) * P, :])
                nc.sync.dma_start(te_g[:, j, :], te_d[t * P:(t + 1) * P, :])
                xT = gp.tile([KC, 3, P], F32, tag="xT")
                xps = ps_a.tile([KC, 3, P], F32, tag="xT_ps")
                for c in range(3):
                    nc.tensor.transpose(xps[:, c, :], x_t[:, c * KC:(c + 1) * KC],
                                        ident[:])
                nc.vector.tensor_copy(xT[:], xps[:])
                ri_ps = ps_a.tile([P, 2 * F], F32, tag="ri_ps")
                for c in range(3):
                    nc.tensor.matmul(ri_ps[:], lhsT=xT[:, c, :], rhs=ccs_sb[:, c, :],
                                     start=(c == 0), stop=(c == 2))
                nc.scalar.activation(ri2[:, j, :], ri_ps[:], AF.Square,
                                     accum_out=ssx_s[:, t:t + 1])

            # ---- stage B: mag + stats. min_max's (mx-mn+1) scale cancels in
            # the l2 normalize, so only mn and the norm are needed:
            #   xn_n = (mag - mn) / sqrt(S2 - 2*mn*S1 + F*mn^2)
            mag = gp.tile([P, G, F], F32, tag="mag")
            nc.gpsimd.tensor_add(mag[:], ri2[:, :, 0:F], ri2[:, :, F:2 * F])
            for j in range(G):
                t = t0 + j
                nc.scalar.activation(mag[:, j, :], mag[:, j, :], AF.Sqrt,
                                     accum_out=mx_s[:, t:t + 1])  # S1 = sum(mag)
            nc.vector.tensor_reduce(mn_s[:, sl], mag[:], axis=AX.X, op=OP.min)
            nc.vector.tensor_reduce(mnt_s[:, sl], te_g[:], axis=AX.X, op=OP.min)
            nc.vector.tensor_reduce(mxt_s[:, sl], te_g[:], axis=AX.X, op=OP.add)
            # nrm2 = S2 - 2*mn*S1 + F*mn^2 ; invs = 1/sqrt(nrm2)
            nc.vector.tensor_mul(rd_s[:, sl], mn_s[:, sl], mx_s[:, sl])
            nc.vector.scalar_tensor_tensor(ssx_s[:, sl], rd_s[:, sl], -2.0,
                                           ssx_s[:, sl], op0=OP.mult, op1=OP.add)
            nc.vector.tensor_mul(rd_s[:, sl], mn_s[:, sl], mn_s[:, sl])
            nc.vector.scalar_tensor_tensor(ssx_s[:, sl], rd_s[:, sl], float(F),
                                           ssx_s[:, sl], op0=OP.mult, op1=OP.add)
            nc.scalar.sqrt(ssx_s[:, sl], ssx_s[:, sl])
            nc.vector.reciprocal(rnx_s[:, sl], ssx_s[:, sl])
            xn_g = gp.tile([P, G, F], F32, tag="xn")
            catg = gp.tile([P, G, H + EMB], F32, tag="catg")
            for j in range(G):
                t = t0 + j
                nc.gpsimd.tensor_scalar(xn_g[:, j, :], mag[:, j, :],
                                        scalar1=mn_s[:, t:t + 1],
                                        scalar2=rnx_s[:, t:t + 1],
                                        op0=OP.subtract, op1=OP.mult)
                scr2 = gp.tile([P, EMB], F32, tag="scrE")
                nc.scalar.activation(scr2[:], te_g[:, j, :], AF.Square,
                                     accum_out=sst_s[:, t:t + 1])
            # te: nrm2t = S2t - 2*mnt*S1t + EMB*mnt^2
            nc.vector.tensor_mul(rdt_s[:, sl], mnt_s[:, sl], mxt_s[:, sl])
            nc.vector.scalar_tensor_tensor(sst_s[:, sl], rdt_s[:, sl], -2.0,
                                           sst_s[:, sl], op0=OP.mult, op1=OP.add)
            nc.vector.tensor_mul(rdt_s[:, sl], mnt_s[:, sl], mnt_s[:, sl])
            nc.vector.scalar_tensor_tensor(sst_s[:, sl], rdt_s[:, sl], float(EMB),
                                           sst_s[:, sl], op0=OP.mult, op1=OP.add)
            nc.scalar.sqrt(sst_s[:, sl], sst_s[:, sl])
            nc.vector.reciprocal(rnt_s[:, sl], sst_s[:, sl])
            for j in range(G):
                t = t0 + j
                nc.gpsimd.tensor_scalar(catg[:, j, H:H + EMB], te_g[:, j, :],
                                        scalar1=mnt_s[:, t:t + 1],
                                        scalar2=rnt_s[:, t:t + 1],
                                        op0=OP.subtract, op1=OP.mult)

            # ---- stage C: q = xn @ Wd0, cat, h = relu(cat @ We0) ----
            hr_g = gp.tile([P, G, H], F32, tag="hr")
            for j in range(G):
                t = t0 + j
                pa = ps_b.tile([P, P], F32, tag="tp_ps")
                nc.tensor.transpose(pa[:], xn_g[:, j, 0:P], ident[:])
                pb = ps_b.tile([F - P, P], F32, tag="tp_ps")
                nc.tensor.transpose(pb[:], xn_g[:, j, P:F], ident[:])
                xnT_a = gp.tile([P, P], F32, tag="xnT_a")
                xnT_b = gp.tile([F - P, P], F32, tag="xnT_b")
                nc.vector.tensor_copy(xnT_a[:], pa[:])
                nc.vector.tensor_copy(xnT_b[:], pb[:])
                q_ps = ps_b.tile([P, H], F32, tag="mm_ps")
                nc.tensor.matmul(q_ps[:], lhsT=xnT_a[:], rhs=wd_a[:], start=True, stop=False)
                nc.tensor.matmul(q_ps[:], lhsT=xnT_b[:], rhs=wd_b[:], start=False, stop=True)
                nc.scalar.activation(catg[:, j, 0:H], q_ps[:], AF.Copy)
                pc = ps_b.tile([H + EMB, P], F32, tag="mm_ps")
                nc.tensor.transpose(pc[:], catg[:, j, :], ident[:])
                catT = gp.tile([H + EMB, P], F32, tag="catT")
                nc.vector.tensor_copy(catT[:], pc[:])
                h_ps = ps_b.tile([P, H], F32, tag="mm_ps")
                nc.tensor.matmul(h_ps[:], lhsT=catT[:], rhs=we_sb[:], start=True, stop=True)
                nc.scalar.activation(hr_g[:, j, :], h_ps[:], AF.Relu,
                                     accum_out=sums_s[:, t:t + 1])

            # ---- stage D: LN + transpose into fp16 hi/lo hT / packed lhsT ----
            nc.vector.tensor_scalar_mul(mean_s[:, sl], sums_s[:, sl], -1.0 / H)
            for j in range(G):
                t = t0 + j
                scr = gp.tile([P, H], F32, tag="scrH")
                nc.scalar.activation(scr[:], hr_g[:, j, :], AF.Square,
                                     bias=mean_s[:, t:t + 1],
                                     accum_out=ssh_s[:, t:t + 1])
            nc.vector.tensor_scalar(ssh_s[:, sl], ssh_s[:, sl], scalar1=1.0 / H,
                                    scalar2=1e-8, op0=OP.mult, op1=OP.add)
            nc.scalar.sqrt(ssh_s[:, sl], ssh_s[:, sl])
            nc.vector.reciprocal(rstd_s[:, sl], ssh_s[:, sl])
            nc.vector.tensor_mul(mnr_s[:, sl], mean_s[:, sl], rstd_s[:, sl])
            for j in range(G):
                t = t0 + j
                h_t = gp.tile([P, H], F32, tag="h_t")
                nc.scalar.activation(h_t[:], hr_g[:, j, :], AF.Identity,
                                     scale=rstd_s[:, t:t + 1],
                                     bias=mnr_s[:, t:t + 1])
                hT_ps = ps_b.tile([H, P], F32, tag="mm_ps")
                nc.tensor.transpose(hT_ps[:], h_t[:], ident[:])
                csl = slice(t * P, (t + 1) * P)
                # hT hi (fp16) and lo = hT - hi (fp16)
                nc.scalar.activation(hTd_sb[0:H, csl], hT_ps[:], AF.Copy)
                nc.vector.scalar_tensor_tensor(hTl_sb[:, csl], hT_ps[:], 1.0,
                                               hTd_sb[0:H, csl],
                                               op0=OP.mult, op1=OP.subtract)
                # lhsT pack: hw_hi into lhsT top half, hw_lo staged on
                # partitions 0-63 (moved to the bottom half by DMA below)
                nc.scalar.activation(lhsT_sb[0:H, t, :], hT_ps[:], AF.Copy,
                                     scale=w2_sb[0:H, 0:1])
                nc.vector.scalar_tensor_tensor(lhsTlo_sb[:, t, :], hT_ps[:],
                                               w2_sb[0:H, 0:1],
                                               lhsT_sb[0:H, t, :],
                                               op0=OP.mult, op1=OP.subtract)

        p1ps.close()

        # move the staged halves onto partitions 64-127 (SBUF->SBUF DMA is
        # the only partition-crossing copy)
        nc.sync.dma_start(hTd_sb[H:2 * H, :], hTd_sb[0:H, :])
        nc.sync.dma_start(lhsT_sb[H:2 * H, :, :], lhsTlo_sb[:, :, :])

        # ---- phase 2: adjacency (fp16 hi/lo split) + quarter top-k ----
        # adj row-tile in PSUM (never relu'd: v10 > 0 makes the final
        # is_ge mask subsume relu). Per 512-col chunk, 2 matmuls:
        #   [hw_hi;hw_lo] @ [hT_hi;hT_hi]  +  hw_hi @ hT_lo
        # Top-10 per row: top-8 of each 512-quarter (exact, fp32) -> 32
        # candidates -> max8/match_replace/max8 on [P,32] -> v10, r.
        # Scale on ACT (PSUM->SBUF, frees PSUM), select on Pool:
        #   out = (adj*r >= v10*r) * (adj*r)
        with tc.tile_pool(name="p2_sb", bufs=3) as p2_sb, \
             tc.tile_pool(name="p2_sm", bufs=6) as p2_sm, \
             tc.tile_pool(name="p2_ps", bufs=8, space="PSUM") as p2_ps:
            for m in range(NT):
                adj_q = []
                mx32 = p2_sm.tile([P, 32], F32, tag="mx32")
                for q in range(4):
                    c0 = q * 512
                    aq = p2_ps.tile([P, 512], F32, tag="adj_q")
                    adj_q.append(aq)
                    nc.tensor.matmul(aq[:], lhsT=lhsT_sb[:, m, :],
                                     rhs=hTd_sb[:, c0:c0 + 512],
                                     start=True, stop=False)
                    nc.tensor.matmul(aq[:], lhsT=lhsT_sb[0:H, m, :],
                                     rhs=hTl_sb[:, c0:c0 + 512],
                                     start=False, stop=True)
                    nc.vector.max(mx32[:, 8 * q:8 * q + 8], aq[:])
                m16 = p2_sm.tile([P, 16], F32, tag="m16")
                z32 = p2_sm.tile([P, 32], F32, tag="z32")
                nc.vector.max(m16[:, 0:8], mx32[:])
                nc.vector.match_replace(z32[:], in_to_replace=m16[:, 0:8],
                                        in_values=mx32[:], imm_value=0.0)
                nc.vector.max(m16[:, 8:16], z32[:])
                # relu the candidates (rows can have <10 positives; the
                # reference top-k runs on relu'd adj); keep the r chain on the
                # DVE queue so the PSUM release is not gated on busy ACT
                nc.vector.tensor_scalar_max(m16[:], m16[:], 0.0)
                den = p2_sm.tile([P, 1], F32, tag="den")
                nc.vector.tensor_reduce(den[:], m16[:, 0:TOPK], axis=AX.X, op=OP.add)
                nc.vector.tensor_scalar_add(den[:], den[:], 1e-5)
                r = p2_sm.tile([P, 1], F32, tag="r")
                nc.vector.reciprocal(r[:], den[:])
                v10r = p2_sm.tile([P, 1], F32, tag="v10r")
                nc.vector.tensor_scalar_mul(v10r[:], m16[:, TOPK - 1:TOPK], r[:, 0:1])
                adj_r = p2_sb.tile([P, N], F32, tag="adj_r")
                for q in range(4):
                    nc.scalar.activation(adj_r[:, 512 * q:512 * q + 512],
                                         adj_q[q][:], AF.Relu, scale=r[:, 0:1])
                outt = p2_sb.tile([P, N], F32, tag="outt")
                # select split across engines: DVE stt on ~42% of columns,
                # Pool is_ge + mult on the rest (stt is DVE-only on hw)
                SPL = 1024
                nc.vector.scalar_tensor_tensor(outt[:, 0:SPL], adj_r[:, 0:SPL],
                                               v10r[:, 0:1], adj_r[:, 0:SPL],
                                               op0=OP.is_ge, op1=OP.mult)
                msk = p2_sb.tile([P, N - SPL], F32, tag="msk")
                nc.gpsimd.tensor_scalar(msk[:], adj_r[:, SPL:N],
                                        scalar1=v10r[:, 0:1], scalar2=None,
                                        op0=OP.is_ge)
                nc.gpsimd.tensor_tensor(outt[:, SPL:N], msk[:],
                                        adj_r[:, SPL:N], op=OP.mult)
                nc.sync.dma_start(out_d[m * P:(m + 1) * P, :], outt[:])

    nc.compile()
    return nc


def _dft_mats():
    tt = np.arange(T)[:, None].astype(np.float64)
    kk = np.arange(F)[None, :].astype(np.float64)
    ang = 2.0 * np.pi * tt * kk / T
    s = 1.0 / np.sqrt(T)
    return (np.cos(ang) * s).astype(np.float32), (np.sin(ang) * s).astype(np.float32)


def kernel(x, t_emb, Wd0, We0, W):
    if "nc" not in _CACHE:
        _CACHE["nc"] = _build()
    nc = _CACHE["nc"]
    cc, cs = _dft_mats()
    base = {
        "ccos": cc, "csin": cs,
        "wd0": np.ascontiguousarray(Wd0, np.float32),
        "we0": np.ascontiguousarray(We0, np.float32),
        "w": np.ascontiguousarray(W, np.float32),
    }
    in_maps = [
        {**base,
         "x": np.ascontiguousarray(x[i], np.float32),
         "t_emb": np.ascontiguousarray(t_emb[i], np.float32)}
        for i in range(NCORES)
    ]
    res = run_bass_kernel_spmd(nc, in_maps, list(range(NCORES)))
    return np.stack([res.results[i]["out"] for i in range(NCORES)], axis=0)



# revision 22
# speedup vs baseline: 1.0324x; 1.0324x over previous
"""Trainium2 Bass kernel for nn_DFDgraph (gnn_message_passing).

Pipeline per batch element (one NeuronCore each, 8 total):
  x (2048, 288) --rfft-mag--> (2048, 145) --minmax+l2--> xn
  h = LN(relu(cat[xn @ Wd0, te_norm] @ We0))            (2048, 64)
  adj = relu((h * w) @ h^T)                             (2048, 2048)
  out = top10_row_mask(adj) / (rowsum_kept + 1e-5)

The rfft is two matmuls against a host-precomputed DFT cos|sin matrix
(288 x 290, ortho-normalized), keeping everything fp32 (f32r loses
~1e-3 relative accuracy, which flips top-k selections). Top-10 per row:
DVE max8 -> match_replace(0) -> max8 gives the 10 largest values
exactly; kept = (adj >= v10) * adj via scalar_tensor_tensor on GpSimd,
final scale 1/(sum_top10 + 1e-5) on ACT.

Phase 1 is processed in groups of 4 row-tiles so the per-group stages
(DFT -> normalize -> MLP -> LN -> transpose) pipeline across groups.
"""

import numpy as np
from contextlib import ExitStack

import concourse.bass as bass
import concourse.mybir as mybir
from concourse import bacc
from concourse import tile
from concourse import masks
from concourse.bass_utils import run_bass_kernel_spmd

F32 = mybir.dt.float32
F16 = mybir.dt.float16
AX = mybir.AxisListType
OP = mybir.AluOpType
AF = mybir.ActivationFunctionType

B, N, T, H, EMB, TOPK = 8, 2048, 288, 64, 24, 10
F = T // 2 + 1          # 145
P = 128                 # rows per tile
NT = N // P             # 16 tiles
G = 4                   # tiles per pipeline group
KC = 96                 # DFT contraction chunk (3 x 96 = 288)
NCORES = 8

_CACHE = {}


def _build(sel_engine="gpsimd"):
    nc = bacc.Bacc("TRN2", target_bir_lowering=False, debug=False,
                   num_devices=NCORES)
    x_d = nc.declare_dram_parameter("x", [N, T], F32, isOutput=False)
    te_d = nc.declare_dram_parameter("t_emb", [N, EMB], F32, isOutput=False)
    cc_d = nc.declare_dram_parameter("ccos", [T, F], F32, isOutput=False)
    cs_d = nc.declare_dram_parameter("csin", [T, F], F32, isOutput=False)
    wd_d = nc.declare_dram_parameter("wd0", [F, H], F32, isOutput=False)
    we_d = nc.declare_dram_parameter("we0", [H + EMB, H], F32, isOutput=False)
    w_d = nc.declare_dram_parameter("w", [H, 1], F32, isOutput=False)
    out_d = nc.declare_dram_parameter("out", [N, N], F32, isOutput=True)

    with tile.TileContext(nc) as tc, ExitStack() as ctx:
        const = ctx.enter_context(tc.tile_pool(name="const", bufs=1))
        ident = const.tile([P, P], F32)
        masks.make_identity(nc, ident[:])
        ccs_sb = const.tile([KC, 3, 2 * F], F32)
        for c in range(3):
            nc.sync.dma_start(ccs_sb[:, c, 0:F], cc_d[c * KC:(c + 1) * KC, :])
            nc.sync.dma_start(ccs_sb[:, c, F:2 * F], cs_d[c * KC:(c + 1) * KC, :])
        wd_a = const.tile([P, H], F32)
        wd_b = const.tile([F - P, H], F32)
        nc.sync.dma_start(wd_a[:], wd_d[0:P, :])
        nc.sync.dma_start(wd_b[:], wd_d[P:F, :])
        we_sb = const.tile([H + EMB, H], F32)
        nc.sync.dma_start(we_sb[:], we_d[:])
        w2_sb = const.tile([H, 1], F32)
        nc.sync.dma_start(w2_sb[:], w_d[:])

        # persistent phase-1 results (fp16 split of h^T for phase-2 matmuls)
        p1 = ctx.enter_context(tc.tile_pool(name="p1", bufs=1))
        hTd_sb = p1.tile([2 * H, N], F16)   # hT hi, duplicated on both halves
        hTl_sb = p1.tile([H, N], F16)       # hT lo
        lhsT_sb = p1.tile([2 * H, NT, P], F16)  # per-tile [hw_hi ; hw_lo]
        lhsTlo_sb = p1.tile([H, NT, P], F16)    # hw_lo staging (parts 0-63)
        # [P, NT] stats, persistent
        st = ctx.enter_context(tc.tile_pool(name="stats", bufs=1))
        mx_s = st.tile([P, NT], F32)
        mn_s = st.tile([P, NT], F32)
        rd_s = st.tile([P, NT], F32)
        ssx_s = st.tile([P, NT], F32)
        rnx_s = st.tile([P, NT], F32)
        mxt_s = st.tile([P, NT], F32)
        mnt_s = st.tile([P, NT], F32)
        rdt_s = st.tile([P, NT], F32)
        sst_s = st.tile([P, NT], F32)
        rnt_s = st.tile([P, NT], F32)
        sums_s = st.tile([P, NT], F32)
        mean_s = st.tile([P, NT], F32)
        ssh_s = st.tile([P, NT], F32)
        rstd_s = st.tile([P, NT], F32)
        mnr_s = st.tile([P, NT], F32)

        # group-cycled working buffers (bufs=2 -> group g+1 overlaps group g)
        p1ps = ExitStack()
        gp = p1ps.enter_context(tc.tile_pool(name="gp", bufs=2))
        ps_a = p1ps.enter_context(tc.tile_pool(name="ps_a", bufs=2, space="PSUM"))
        ps_b = p1ps.enter_context(tc.tile_pool(name="ps_b", bufs=2, space="PSUM"))

        for g in range(NT // G):
            t0 = g * G
            sl = slice(t0, t0 + G)
            # ---- stage A: load x/te, transpose, DFT, squares ----
            # ri2 = [re^2 | im^2]; its ACT accum gives S2 = sum(mag^2) free
            ri2 = gp.tile([P, G, 2 * F], F32, tag="ri2")
            te_g = gp.tile([P, G, EMB], F32, tag="te")
            for j in range(G):
                t = t0 + j
                x_t = gp.tile([P, T], F32, tag="x")
                nc.sync.dma_start(x_t[:], x_d[t * P:(t + 1) * P, :])
                nc.sync.dma_start(te_g[:, j, :], te_d[t * P:(t + 1) * P, :])
                xT = gp.tile([KC, 3, P], F32, tag="xT")
                xps = ps_a.tile([KC, 3, P], F32, tag="xT_ps")
                for c in range(3):
                    nc.tensor.transpose(xps[:, c, :], x_t[:, c * KC:(c + 1) * KC],
                                        ident[:])
                nc.vector.tensor_copy(xT[:], xps[:])
                ri_ps = ps_a.tile([P, 2 * F], F32, tag="ri_ps")
                for c in range(3):
                    nc.tensor.matmul(ri_ps[:], lhsT=xT[:, c, :], rhs=ccs_sb[:, c, :],
                                     start=(c == 0), stop=(c == 2))
                nc.scalar.activation(ri2[:, j, :], ri_ps[:], AF.Square,
                                     accum_out=ssx_s[:, t:t + 1])

            # ---- stage B: mag + stats. min_max's (mx-mn+1) scale cancels in
            # the l2 normalize, so only mn and the norm are needed:
            #   xn_n = (mag - mn) / sqrt(S2 - 2*mn*S1 + F*mn^2)
            mag = gp.tile([P, G, F], F32, tag="mag")
            nc.gpsimd.tensor_add(mag[:], ri2[:, :, 0:F], ri2[:, :, F:2 * F])
            for j in range(G):
                t = t0 + j
                nc.scalar.activation(mag[:, j, :], mag[:, j, :], AF.Sqrt,
                                     accum_out=mx_s[:, t:t + 1])  # S1 = sum(mag)
            nc.vector.tensor_reduce(mn_s[:, sl], mag[:], axis=AX.X, op=OP.min)
            nc.vector.tensor_reduce(mnt_s[:, sl], te_g[:], axis=AX.X, op=OP.min)
            nc.vector.tensor_reduce(mxt_s[:, sl], te_g[:], axis=AX.X, op=OP.add)
            # nrm2 = S2 - 2*mn*S1 + F*mn^2 ; invs = 1/sqrt(nrm2)
            nc.vector.tensor_mul(rd_s[:, sl], mn_s[:, sl], mx_s[:, sl])
            nc.vector.scalar_tensor_tensor(ssx_s[:, sl], rd_s[:, sl], -2.0,
                                           ssx_s[:, sl], op0=OP.mult, op1=OP.add)
            nc.vector.tensor_mul(rd_s[:, sl], mn_s[:, sl], mn_s[:, sl])
            nc.vector.scalar_tensor_tensor(ssx_s[:, sl], rd_s[:, sl], float(F),
                                           ssx_s[:, sl], op0=OP.mult, op1=OP.add)
            nc.scalar.sqrt(ssx_s[:, sl], ssx_s[:, sl])
            nc.vector.reciprocal(rnx_s[:, sl], ssx_s[:, sl])
            xn_g = gp.tile([P, G, F], F32, tag="xn")
            catg = gp.tile([P, G, H + EMB], F32, tag="catg")
            for j in range(G):
                t = t0 + j
                nc.gpsimd.tensor_scalar(xn_g[:, j, :], mag[:, j, :],
                                        scalar1=mn_s[:, t:t + 1],
                                        scalar2=rnx_s[:, t:t + 1],
                                        op0=OP.subtract, op1=OP.mult)
                scr2 = gp.tile([P, EMB], F32, tag="scrE")
                nc.scalar.activation(scr2[:], te_g[:, j, :], AF.Square,
                                     accum_out=sst_s[:, t:t + 1])
            # te: nrm2t = S2t - 2*mnt*S1t + EMB*mnt^2
            nc.vector.tensor_mul(rdt_s[:, sl], mnt_s[:, sl], mxt_s[:, sl])
            nc.vector.scalar_tensor_tensor(sst_s[:, sl], rdt_s[:, sl], -2.0,
                                           sst_s[:, sl], op0=OP.mult, op1=OP.add)
            nc.vector.tensor_mul(rdt_s[:, sl], mnt_s[:, sl], mnt_s[:, sl])
            nc.vector.scalar_tensor_tensor(sst_s[:, sl], rdt_s[:, sl], float(EMB),
                                           sst_s[:, sl], op0=OP.mult, op1=OP.add)
            nc.scalar.sqrt(sst_s[:, sl], sst_s[:, sl])
            nc.vector.reciprocal(rnt_s[:, sl], sst_s[:, sl])
            for j in range(G):
                t = t0 + j
                nc.gpsimd.tensor_scalar(catg[:, j, H:H + EMB], te_g[:, j, :],
                                        scalar1=mnt_s[:, t:t + 1],
                                        scalar2=rnt_s[:, t:t + 1],
                                        op0=OP.subtract, op1=OP.mult)

            # ---- stage C: q = xn @ Wd0, cat, h = relu(cat @ We0) ----
            hr_g = gp.tile([P, G, H], F32, tag="hr")
            for j in range(G):
                t = t0 + j
                pa = ps_b.tile([P, P], F32, tag="tp_ps")
                nc.tensor.transpose(pa[:], xn_g[:, j, 0:P], ident[:])
                pb = ps_b.tile([F - P, P], F32, tag="tp_ps")
                nc.tensor.transpose(pb[:], xn_g[:, j, P:F], ident[:])
                xnT_a = gp.tile([P, P], F32, tag="xnT_a")
                xnT_b = gp.tile([F - P, P], F32, tag="xnT_b")
                nc.vector.tensor_copy(xnT_a[:], pa[:])
                nc.vector.tensor_copy(xnT_b[:], pb[:])
                q_ps = ps_b.tile([P, H], F32, tag="mm_ps")
                nc.tensor.matmul(q_ps[:], lhsT=xnT_a[:], rhs=wd_a[:], start=True, stop=False)
                nc.tensor.matmul(q_ps[:], lhsT=xnT_b[:], rhs=wd_b[:], start=False, stop=True)
                nc.scalar.activation(catg[:, j, 0:H], q_ps[:], AF.Copy)
                pc = ps_b.tile([H + EMB, P], F32, tag="mm_ps")
                nc.tensor.transpose(pc[:], catg[:, j, :], ident[:])
                catT = gp.tile([H + EMB, P], F32, tag="catT")
                nc.vector.tensor_copy(catT[:], pc[:])
                h_ps = ps_b.tile([P, H], F32, tag="mm_ps")
                nc.tensor.matmul(h_ps[:], lhsT=catT[:], rhs=we_sb[:], start=True, stop=True)
                nc.scalar.activation(hr_g[:, j, :], h_ps[:], AF.Relu,
                                     accum_out=sums_s[:, t:t + 1])

            # ---- stage D: LN + transpose into fp16 hi/lo hT / packed lhsT ----
            nc.vector.tensor_scalar_mul(mean_s[:, sl], sums_s[:, sl], -1.0 / H)
            for j in range(G):
                t = t0 + j
                scr = gp.tile([P, H], F32, tag="scrH")
                nc.scalar.activation(scr[:], hr_g[:, j, :], AF.Square,
                                     bias=mean_s[:, t:t + 1],
                                     accum_out=ssh_s[:, t:t + 1])
            nc.vector.tensor_scalar(ssh_s[:, sl], ssh_s[:, sl], scalar1=1.0 / H,
                                    scalar2=1e-8, op0=OP.mult, op1=OP.add)
            nc.scalar.sqrt(ssh_s[:, sl], ssh_s[:, sl])
            nc.vector.reciprocal(rstd_s[:, sl], ssh_s[:, sl])
            nc.vector.tensor_mul(mnr_s[:, sl], mean_s[:, sl], rstd_s[:, sl])
            for j in range(G):
                t = t0 + j
                h_t = gp.tile([P, H], F32, tag="h_t")
                nc.scalar.activation(h_t[:], hr_g[:, j, :], AF.Identity,
                                     scale=rstd_s[:, t:t + 1],
                                     bias=mnr_s[:, t:t + 1])
                hT_ps = ps_b.tile([H, P], F32, tag="mm_ps")
                nc.tensor.transpose(hT_ps[:], h_t[:], ident[:])
                csl = slice(t * P, (t + 1) * P)
                # hT hi (fp16) and lo = hT - hi (fp16)
                nc.scalar.activation(hTd_sb[0:H, csl], hT_ps[:], AF.Copy)
                nc.vector.scalar_tensor_tensor(hTl_sb[:, csl], hT_ps[:], 1.0,
                                               hTd_sb[0:H, csl],
                                               op0=OP.mult, op1=OP.subtract)
                # lhsT pack: hw_hi into lhsT top half, hw_lo staged on
                # partitions 0-63 (moved to the bottom half by DMA below)
                nc.scalar.activation(lhsT_sb[0:H, t, :], hT_ps[:], AF.Copy,
                                     scale=w2_sb[0:H, 0:1])
                nc.vector.scalar_tensor_tensor(lhsTlo_sb[:, t, :], hT_ps[:],
                                               w2_sb[0:H, 0:1],
                                               lhsT_sb[0:H, t, :],
                                               op0=OP.mult, op1=OP.subtract)

        p1ps.close()

        # move the staged halves onto partitions 64-127 (SBUF->SBUF DMA is
        # the only partition-crossing copy)
        nc.sync.dma_start(hTd_sb[H:2 * H, :], hTd_sb[0:H, :])
        nc.sync.dma_start(lhsT_sb[H:2 * H, :, :], lhsTlo_sb[:, :, :])

        # ---- phase 2: adjacency (fp16 hi/lo split) + quarter top-k ----
        # adj row-tile in PSUM (never relu'd: v10 > 0 makes the final
        # is_ge mask subsume relu). Per 512-col chunk, 2 matmuls:
        #   [hw_hi;hw_lo] @ [hT_hi;hT_hi]  +  hw_hi @ hT_lo
        # Top-10 per row: top-8 of each 512-quarter (exact, fp32) -> 32
        # candidates -> max8/match_replace/max8 on [P,32] -> v10, r.
        # Scale on ACT (PSUM->SBUF, frees PSUM), select on Pool:
        #   out = (adj*r >= v10*r) * (adj*r)
        with tc.tile_pool(name="p2_sb", bufs=3) as p2_sb, \
             tc.tile_pool(name="p2_sm", bufs=6) as p2_sm, \
             tc.tile_pool(name="p2_ps", bufs=8, space="PSUM") as p2_ps:
            for m in range(NT):
                adj_q = []
                mx32 = p2_sm.tile([P, 32], F32, tag="mx32")
                for q in range(4):
                    c0 = q * 512
                    aq = p2_ps.tile([P, 512], F32, tag="adj_q")
                    adj_q.append(aq)
                    nc.tensor.matmul(aq[:], lhsT=lhsT_sb[:, m, :],
                                     rhs=hTd_sb[:, c0:c0 + 512],
                                     start=True, stop=False)
                    nc.tensor.matmul(aq[:], lhsT=lhsT_sb[0:H, m, :],
                                     rhs=hTl_sb[:, c0:c0 + 512],
                                     start=False, stop=True)
                    nc.vector.max(mx32[:, 8 * q:8 * q + 8], aq[:])
                m16 = p2_sm.tile([P, 16], F32, tag="m16")
                z32 = p2_sm.tile([P, 32], F32, tag="z32")
                nc.vector.max(m16[:, 0:8], mx32[:])
                nc.vector.match_replace(z32[:], in_to_replace=m16[:, 0:8],
                                        in_values=mx32[:], imm_value=0.0)
                nc.vector.max(m16[:, 8:16], z32[:])
                # relu the candidates (rows can have <10 positives; the
                # reference top-k runs on relu'd adj) and row-sum them in one
                # ACT op: den = sum(relu(top10)), m10 = relu'd top10
                m10 = p2_sm.tile([P, TOPK], F32, tag="m10")
                den = p2_sm.tile([P, 1], F32, tag="den")
                nc.scalar.activation(m10[:], m16[:, 0:TOPK], AF.Relu,
                                     accum_out=den[:])
                nc.vector.tensor_scalar_add(den[:], den[:], 1e-5)
                r = p2_sm.tile([P, 1], F32, tag="r")
                nc.vector.reciprocal(r[:], den[:])
                v10r = p2_sm.tile([P, 1], F32, tag="v10r")
                nc.vector.tensor_scalar_mul(v10r[:], m10[:, TOPK - 1:TOPK], r[:, 0:1])
                adj_r = p2_sb.tile([P, N], F32, tag="adj_r")
                for q in range(4):
                    nc.scalar.activation(adj_r[:, 512 * q:512 * q + 512],
                                         adj_q[q][:], AF.Relu, scale=r[:, 0:1])
                outt = p2_sb.tile([P, N], F32, tag="outt")
                # select split across engines: DVE stt on ~42% of columns,
                # Pool is_ge + mult on the rest (stt is DVE-only on hw)
                SPL = 864
                nc.vector.scalar_tensor_tensor(outt[:, 0:SPL], adj_r[:, 0:SPL],
                                               v10r[:, 0:1], adj_r[:, 0:SPL],
                                               op0=OP.is_ge, op1=OP.mult)
                msk = p2_sb.tile([P, N - SPL], F32, tag="msk")
                nc.gpsimd.tensor_scalar(msk[:], adj_r[:, SPL:N],
                                        scalar1=v10r[:, 0:1], scalar2=None,
                                        op0=OP.is_ge)
                nc.gpsimd.tensor_tensor(outt[:, SPL:N], msk[:],
                                        adj_r[:, SPL:N], op=OP.mult)
                nc.sync.dma_start(out_d[m * P:(m + 1) * P, :], outt[:])

    nc.compile()
    return nc


def _dft_mats():
    tt = np.arange(T)[:, None].astype(np.float64)
    kk = np.arange(F)[None, :].astype(np.float64)
    ang = 2.0 * np.pi * tt * kk / T
    s = 1.0 / np.sqrt(T)
    return (np.cos(ang) * s).astype(np.float32), (np.sin(ang) * s).astype(np.float32)


def kernel(x, t_emb, Wd0, We0, W):
    if "nc" not in _CACHE:
        _CACHE["nc"] = _build()
    nc = _CACHE["nc"]
    cc, cs = _dft_mats()
    base = {
        "ccos": cc, "csin": cs,
        "wd0": np.ascontiguousarray(Wd0, np.float32),
        "we0": np.ascontiguousarray(We0, np.float32),
        "w": np.ascontiguousarray(W, np.float32),
    }
    in_maps = [
        {**base,
         "x": np.ascontiguousarray(x[i], np.float32),
         "t_emb": np.ascontiguousarray(t_emb[i], np.float32)}
        for i in range(NCORES)
    ]
    res = run_bass_kernel_spmd(nc, in_maps, list(range(NCORES)))
    return np.stack([res.results[i]["out"] for i in range(NCORES)], axis=0)

